# revision 2
# baseline (speedup 1.0000x reference)
# Trainium2 Bass kernel for CoAttentionModule (axial co-attention, 8 heads).
#
# Sharding: data-parallel over (direction, batch) = 2 x 4 = 8 NeuronCores.
# Core c computes weighted = _coattention(qf, rf)[b].T for its (d, b) pair;
# the host concatenates [features, weighted] per direction.
#
# On-chip layout: the hw axis is w-major everywhere (col = w*32 + i, i = h
# index); the host pre-permutes features and un-permutes the output. This
# makes every matmul stationary operand a contiguous SBUF slice (walrus
# requires single-free-dim weight APs).
#
# Per-core pipeline (bf16 matmul operands, fp32 PSUM accumulation):
#   qT = Wq.T @ xq (+bq)          [c_out, hw]
#   kT = Wk.T @ xr  + RWF         RWF[c,(w,k)] = rel_emb[(k-w)%63, c]  (rel_w
#                                 term folded into keys; bk cancels in softmax)
#   v  = xr.T @ Wv                [(w,k), c]
#   QAUG[t', col(w,i)] = sum_c rel_emb[(t'-i)%63, c] q[c, col]  (63 aug rows,
#                                 via 32 host-rolled copies of rel_emb.T)
#   scores tile (head n, w-group of 4) [128=(w,i), 128=(w,k)]:
#       q.k' + QAUG.KAUG(one-hot) + WIND.KMASK(-1e30 off-diag mask channels)
#   softmax: exp(scale=1/16) with accum_out row sums -> reciprocal -> scale
#   probsT via DVE 32x32 stream transpose (block-diagonal => exact transpose)
#   avT[c,(w,i)] = v.T @ probsT ; output proj outT = Wo.T @ attT + bo'
#   (bv folded on host: bo' = bv @ Wo + bo; bk dropped: softmax-invariant)
import numpy as np
import ml_dtypes

B, C, H, W = 4, 2048, 32, 32
HW = H * W
NH, HD = 8, 256
T = 2 * max(H, W) - 1  # 63
NC = C // 128  # 16 chunks

_CACHE = {}


def _hostprep(Wq, bq, Wk, bk, Wv, bv, Wo, bo, rel_emb):
    bf = ml_dtypes.bfloat16
    f32 = np.float32
    Wq, Wk, Wv, Wo = (np.asarray(a, f32) for a in (Wq, Wk, Wv, Wo))
    rel = np.asarray(rel_emb, f32)  # [63, 256]
    ii = np.arange(32)

    # lhsT chunk-major weight blobs [co, ci, 128, 128]
    def lchunks(Wm):
        return np.ascontiguousarray(
            Wm.reshape(NC, 128, NC, 128).transpose(2, 0, 1, 3)
        ).astype(bf)

    wq_l = lchunks(Wq)
    wk_l = lchunks(Wk)
    wo_l = lchunks(Wo)
    # V weights as rhs chunks per head: [n, ci, 128, 256]
    wv_r = np.ascontiguousarray(
        Wv.reshape(NC, 128, NH, HD).transpose(2, 0, 1, 3)
    ).astype(bf)

    bq_c = np.ascontiguousarray(np.asarray(bq, f32).reshape(NC, 128).T)  # [128,16]
    bo2 = np.asarray(bv, f32) @ Wo + np.asarray(bo, f32)
    bo2_c = np.ascontiguousarray(bo2.reshape(NC, 128).T)  # [128,16]

    w_idx, k_idx = np.meshgrid(np.arange(32), np.arange(32), indexing="ij")
    # rel_w fold table, w-major [2, 128, 1024]: rwf[ch, p, w*32+k] = rel[(k-w)%63, ch*128+p]
    rwf = rel[(k_idx - w_idx) % T].reshape(HW, HD)  # [(w,k), 256]
    rwf = np.ascontiguousarray(rwf.T.reshape(2, 128, HW)).astype(f32)
    # rolled rel_emb.T for QAUG: relroll[p, (i, ch, t')] = rel[(t'-i)%63, ch*128+p]
    relroll = np.zeros((128, 32 * 2 * T), f32)
    for i in range(32):
        for ch in range(2):
            blk = rel[(np.arange(T) - i) % T, ch * 128:(ch + 1) * 128]  # [63,128]
            relroll[:, (i * 2 + ch) * T:(i * 2 + ch + 1) * T] = blk.T
    relroll = relroll.astype(bf)
    # one-hot key-aug [63, 1024] w-major: kaug[t, w*32+k] = (t == k)
    kaug = np.zeros((T, HW), f32)
    kaug[k_idx.reshape(-1), np.arange(HW)] = 1.0
    kaug = kaug.astype(bf)
    # mask channels [32, 1024] w-major
    wind = np.zeros((32, HW), f32)
    kmask = np.full((32, HW), -1e30, f32)
    for w in range(32):
        wind[w, w * 32 + ii] = 1.0  # query col w*32+i
        kmask[w, w * 32 + ii] = 0.0  # key col w*32+k
    wind = wind.astype(bf)
    kmask = kmask.astype(bf)

    return dict(wq_l=wq_l, wk_l=wk_l, wo_l=wo_l, wv_r=wv_r, bq_c=bq_c,
                bo2_c=bo2_c, rwf=rwf, relroll=relroll, kaug=kaug, wind=wind,
                kmask=kmask)


def _build():
    import concourse.bacc as bacc
    import concourse.mybir as mybir
    import concourse.tile as tile

    F32, BF16 = mybir.dt.float32, mybir.dt.bfloat16
    nc = bacc.Bacc(None, target_bir_lowering=False)

    xq = nc.declare_dram_parameter("xq", [C, HW], BF16, isOutput=False)
    xr = nc.declare_dram_parameter("xr", [C, HW], BF16, isOutput=False)
    wq_l = nc.declare_dram_parameter("wq_l", [NC, NC, 128, 128], BF16, isOutput=False)
    wk_l = nc.declare_dram_parameter("wk_l", [NC, NC, 128, 128], BF16, isOutput=False)
    wo_l = nc.declare_dram_parameter("wo_l", [NC, NC, 128, 128], BF16, isOutput=False)
    wv_r = nc.declare_dram_parameter("wv_r", [NH, NC, 128, HD], BF16, isOutput=False)
    bq_c = nc.declare_dram_parameter("bq_c", [128, NC], F32, isOutput=False)
    bo2_c = nc.declare_dram_parameter("bo2_c", [128, NC], F32, isOutput=False)
    rwf = nc.declare_dram_parameter("rwf", [2, 128, HW], F32, isOutput=False)
    relroll = nc.declare_dram_parameter("relroll", [128, 32 * 2 * T], BF16, isOutput=False)
    kaug = nc.declare_dram_parameter("kaug", [T, HW], BF16, isOutput=False)
    wind = nc.declare_dram_parameter("wind", [32, HW], BF16, isOutput=False)
    kmask = nc.declare_dram_parameter("kmask", [32, HW], BF16, isOutput=False)
    out = nc.declare_dram_parameter("out", [C, HW], F32, isOutput=True)

    EXP = mybir.ActivationFunctionType.Exp

    with tile.TileContext(nc) as tc:
        with (
            tc.tile_pool(name="feat", bufs=2) as feat_pool,
            tc.tile_pool(name="att", bufs=1) as att_pool,
            tc.tile_pool(name="const", bufs=1) as const_pool,
            tc.tile_pool(name="head", bufs=2) as head_pool,
            tc.tile_pool(name="wstr", bufs=4) as wstr_pool,
            tc.tile_pool(name="probs", bufs=3) as probs_pool,
            tc.tile_pool(name="outs", bufs=3) as outs_pool,
            tc.tile_pool(name="psum", bufs=2, space="PSUM") as psum_pool,
        ):
            # ---- load features + constants (resident) ----
            xqt = feat_pool.tile([128, NC * HW], BF16, tag="feat")
            xrt = feat_pool.tile([128, NC * HW], BF16, tag="feat")
            for cc in range(NC):
                nc.sync.dma_start(xqt[:, cc * HW:(cc + 1) * HW], xq[cc * 128:(cc + 1) * 128, :])
                nc.sync.dma_start(xrt[:, cc * HW:(cc + 1) * HW], xr[cc * 128:(cc + 1) * 128, :])
            attT = att_pool.tile([128, NC * HW], BF16)

            c_kaug = const_pool.tile([T, HW], BF16)
            nc.sync.dma_start(c_kaug[:], kaug[:])
            c_wind = const_pool.tile([32, HW], BF16)
            nc.sync.dma_start(c_wind[:], wind[:])
            c_kmask = const_pool.tile([32, HW], BF16)
            nc.sync.dma_start(c_kmask[:], kmask[:])
            c_rwf = const_pool.tile([128, 2 * HW], F32)
            nc.sync.dma_start(c_rwf[:, 0:HW], rwf[0])
            nc.sync.dma_start(c_rwf[:, HW:2 * HW], rwf[1])
            c_roll = const_pool.tile([128, 32 * 2 * T], BF16)
            nc.sync.dma_start(c_roll[:], relroll[:])
            c_bq = const_pool.tile([128, NC], F32)
            nc.sync.dma_start(c_bq[:], bq_c[:])
            c_bo = const_pool.tile([128, NC], F32)
            nc.sync.dma_start(c_bo[:], bo2_c[:])

            for n in range(NH):
                sq = head_pool.tile([128, 2 * HW], BF16, tag="sq")
                sk = head_pool.tile([128, 2 * HW], BF16, tag="sk")
                sv = head_pool.tile([128, NH * HD], BF16, tag="sv")
                sqa = head_pool.tile([T, HW], BF16, tag="sqa")
                swv = head_pool.tile([128, NC * HD], BF16, tag="swv")

                # stage this head's V weights once
                for ci in range(NC):
                    nc.sync.dma_start(swv[:, ci * HD:(ci + 1) * HD], wv_r[n, ci])

                # ---- Q / K projections: psum[co2,h2] = sum_ci W.T @ x ----
                for which in range(2):  # 0 = Q, 1 = K
                    wsrc = wq_l if which == 0 else wk_l
                    xsrc = xqt if which == 0 else xrt
                    dst = sq if which == 0 else sk
                    for co2 in range(2):
                        co = n * 2 + co2
                        for h2 in range(2):
                            ps = psum_pool.tile([128, 512], F32, tag="pp")
                            for ci in range(NC):
                                wt = wstr_pool.tile([128, 128], BF16, tag="wl")
                                nc.sync.dma_start(wt[:], wsrc[co, ci])
                                nc.tensor.matmul(
                                    ps[:], wt[:],
                                    xsrc[:, ci * HW + h2 * 512: ci * HW + (h2 + 1) * 512],
                                    start=(ci == 0), stop=(ci == NC - 1))
                            dpos = dst[:, co2 * HW + h2 * 512: co2 * HW + (h2 + 1) * 512]
                            if which == 0:
                                nc.vector.tensor_scalar_add(dpos, ps[:], c_bq[:, co:co + 1])
                            else:
                                nc.vector.tensor_add(
                                    dpos, ps[:],
                                    c_rwf[:, co2 * HW + h2 * 512: co2 * HW + (h2 + 1) * 512])

                # ---- V projection, w-major rows ----
                for wg in range(8):
                    psv = psum_pool.tile([128, HD], F32, tag="pp")
                    for ci in range(NC):
                        nc.tensor.matmul(
                            psv[:], xrt[:, ci * HW + wg * 128: ci * HW + (wg + 1) * 128],
                            swv[:, ci * HD:(ci + 1) * HD],
                            start=(ci == 0), stop=(ci == NC - 1))
                    nc.vector.tensor_copy(sv[:, wg * HD:(wg + 1) * HD], psv[:])

                # ---- QAUG: per query-row i, rolled rel_emb.T contraction ----
                for half in range(2):
                    pqa = psum_pool.tile([T, 512], F32, tag="qa")
                    for io in range(16):
                        i = half * 16 + io
                        for ch in range(2):
                            nc.tensor.matmul(
                                pqa[:, io * 32:(io + 1) * 32],
                                c_roll[:, (i * 2 + ch) * T:(i * 2 + ch + 1) * T],
                                sq[:, ch * HW + i: (ch + 1) * HW: 32],
                                start=(ch == 0), stop=(ch == 1))
                    # pqa cols are (i, w) pairs; sqa is w-major (w*32+i)
                    nc.vector.tensor_copy(
                        sqa.rearrange("p (w i) -> p i w", i=32)[:, half * 16:(half + 1) * 16, :],
                        pqa[:].rearrange("p (i w) -> p i w", w=32))

                # ---- attention per w-group ----
                for wg in range(8):
                    sc = psum_pool.tile([128, 128], F32, tag="sc")
                    nc.tensor.matmul(sc[:], sq[:, wg * 128:(wg + 1) * 128],
                                     sk[:, wg * 128:(wg + 1) * 128],
                                     start=True, stop=False)
                    nc.tensor.matmul(sc[:], sq[:, HW + wg * 128: HW + (wg + 1) * 128],
                                     sk[:, HW + wg * 128: HW + (wg + 1) * 128],
                                     start=False, stop=False)
                    nc.tensor.matmul(sc[:], sqa[:, wg * 128:(wg + 1) * 128],
                                     c_kaug[:, wg * 128:(wg + 1) * 128],
                                     start=False, stop=False)
                    nc.tensor.matmul(sc[:], c_wind[:, wg * 128:(wg + 1) * 128],
                                     c_kmask[:, wg * 128:(wg + 1) * 128],
                                     start=False, stop=True)
                    probs = probs_pool.tile([128, 128], BF16, tag="pr")
                    sums = probs_pool.tile([128, 1], F32, tag="sm")
                    recip = probs_pool.tile([128, 1], F32, tag="rc")
                    nc.scalar.activation(probs[:], sc[:], EXP, scale=1.0 / 16.0,
                                         accum_out=sums[:])
                    nc.vector.reciprocal(recip[:], sums[:])
                    nc.vector.tensor_scalar_mul(probs[:], probs[:], recip[:])
                    probsT = probs_pool.tile([128, 128], BF16, tag="prT")
                    nc.vector.transpose(probsT[:], probs[:])
                    for ch in range(2):
                        av = psum_pool.tile([128, 128], F32, tag="av")
                        nc.tensor.matmul(
                            av[:], sv[:, wg * HD + ch * 128: wg * HD + (ch + 1) * 128],
                            probsT[:], start=True, stop=True)
                        nc.vector.tensor_copy(
                            attT[:, (n * 2 + ch) * HW + wg * 128:
                                 (n * 2 + ch) * HW + (wg + 1) * 128], av[:])

            # ---- output projection ----
            for co in range(NC):
                for h2 in range(2):
                    ps = psum_pool.tile([128, 512], F32, tag="pp")
                    for ci in range(NC):
                        wt = wstr_pool.tile([128, 128], BF16, tag="wl")
                        nc.sync.dma_start(wt[:], wo_l[co, ci])
                        nc.tensor.matmul(
                            ps[:], wt[:],
                            attT[:, ci * HW + h2 * 512: ci * HW + (h2 + 1) * 512],
                            start=(ci == 0), stop=(ci == NC - 1))
                    ot = outs_pool.tile([128, 512], F32, tag="ot")
                    nc.vector.tensor_scalar_add(ot[:], ps[:], c_bo[:, co:co + 1])
                    nc.sync.dma_start(
                        out[co * 128:(co + 1) * 128, h2 * 512:(h2 + 1) * 512], ot[:])

    nc.finalize()
    return nc


def kernel(left_features, right_features, Wq, bq, Wk, bk, Wv, bv, Wo, bo, rel_emb,
           _trace=False):
    from concourse.bass_utils import run_bass_kernel_spmd

    bf = ml_dtypes.bfloat16
    if "nc" not in _CACHE:
        _CACHE["nc"] = _build()
    nc = _CACHE["nc"]

    consts = _hostprep(Wq, bq, Wk, bk, Wv, bv, Wo, bo, rel_emb)
    lf = np.asarray(left_features, np.float32)
    rf = np.asarray(right_features, np.float32)

    def wmajor(x):  # (C, H, W) -> (C, HW) with col = w*32 + i
        return np.ascontiguousarray(x.transpose(0, 2, 1).reshape(C, HW)).astype(bf)

    in_maps = []
    for core in range(8):
        d, b = divmod(core, 4)
        qf = lf[b] if d == 0 else rf[b]
        rfb = rf[b] if d == 0 else lf[b]
        m = dict(consts)
        m["xq"] = wmajor(qf)
        m["xr"] = wmajor(rfb)
        in_maps.append(m)

    res = run_bass_kernel_spmd(nc, in_maps, list(range(8)), trace=_trace)
    _CACHE["last_result"] = res

    def unperm(o):  # [C, HW w-major] -> (C, H, W)
        return np.ascontiguousarray(o.reshape(C, W, H).transpose(0, 2, 1))

    wr = np.stack([unperm(res.results[b]["out"]) for b in range(4)])
    wl = np.stack([unperm(res.results[4 + b]["out"]) for b in range(4)])
    left_att = np.concatenate([lf, wr], axis=1)
    right_att = np.concatenate([rf, wl], axis=1)
    return (left_att, right_att)
